# revision 13
# baseline (speedup 1.0000x reference)
"""DGCNN (nn_DGCNN_70643622084664) Trainium2 Bass kernel.

Data-parallel over batch: B=16 samples sharded 2-per-core across 8 NeuronCores.
Per sample pipeline (all on device):
  stage A: pairwise -dist^2 ranking matrix via augmented matmul -> top-10 via
           DVE max8/max_index/match_replace -> gpsimd ap_gather of conv1-L1
           pre-activations -> +v, LeakyReLU -> conv1-L2 matmul -> max over k
  stage B: same with C=22 -> x2
  stage C: conv3 (two 1x1 conv layers) on [x1;x2] -> global max over N
  stage D: MLP + log_softmax

BN (eval, running stats 0/1) is folded into weights/biases on the host.
The first layer of each edge-conv is decomposed as
  W @ [feat-center; center] = u_gather + v,  u = Wf' x,  v = (Wc'-Wf') x + b'
so the k-NN gather happens after the (cheap) pointwise conv.
"""

import functools
import os

SCAN_V2 = bool(int(os.environ.get("KERNEL_SCANV2", "1")))

import numpy as np

EPS = 1e-5
SLOPE = 0.2
KNN = 10
NEG_BIG = -3.0e38

# ---------------------------------------------------------------------------
# program builder
# ---------------------------------------------------------------------------


@functools.lru_cache(maxsize=None)
def _build_program(nsamp: int, n_pts: int, scan_v2: bool = True):
    import concourse.bacc as bacc
    import concourse.bass as bass
    import concourse.mybir as mybir
    from concourse import tile

    f32 = mybir.dt.float32
    f16 = mybir.dt.float16
    u16 = mybir.dt.uint16
    i16 = mybir.dt.int16
    Alu = mybir.AluOpType
    Act = mybir.ActivationFunctionType

    N = n_pts
    NT = N // 128
    NS = N // 16          # gather-list slots per kk
    FC = N // 512 if N >= 512 else 1   # 512-wide free chunks per full row
    FCW = min(512, N)

    nc = bacc.Bacc("TRN2", target_bir_lowering=False, debug=False)

    def din(name, shape, dt=f32):
        return nc.dram_tensor(name, list(shape), dt, kind="ExternalInput").ap()

    # inputs -----------------------------------------------------------------
    x_d = din("x", [nsamp, 2, N])
    ones_col = din("ones_col", [128, 1])
    w1u = din("w1u", [2, 128])          # conv1-L1 u lhsT (replicated x8)
    w1v = din("w1v", [3, 128])          # conv1-L1 v lhsT (+bias row)
    w1b2 = din("w1b2", [16, 22])        # conv1-L2 lhsT  (fp32)
    b1b = din("b1b", [22, 1])
    w2u = din("w2u", [22, 112])
    w2v = din("w2v", [23, 112])
    w2b2 = din("w2b2", [112, 256], f16)  # conv2-L2 lhsT, M split 2x128
    b2b = din("b2b", [128, 2])
    w3a0 = din("w3a0", [22, 266], f16)   # conv3-L1 lhsT K-tiles (x1 | x2a | x2b)
    w3a1 = din("w3a1", [128, 266], f16)
    w3a2 = din("w3a2", [128, 266], f16)
    b3a = din("b3a", [128, 3])
    w3b0 = din("w3b0", [128, 256], f16)  # conv3-L2 lhsT K-tiles
    w3b1 = din("w3b1", [128, 256], f16)
    w3b2 = din("w3b2", [10, 256], f16)
    b3b = din("b3b", [128, 2])
    wl10 = din("wl10", [128, 148], f16)
    wl11 = din("wl11", [128, 148], f16)
    bl1 = din("bl1", [128, 2])
    wl20 = din("wl20", [128, 40], f16)
    wl21 = din("wl21", [20, 40], f16)
    bl2 = din("bl2", [40, 1])

    out_d = nc.dram_tensor("out", [nsamp, 40], f32, kind="ExternalOutput").ap()

    w3a_k = [(w3a0, 22), (w3a1, 128), (w3a2, 128)]
    w3b_k = [(w3b0, 128), (w3b1, 128), (w3b2, 10)]
    m3a = [(0, 128), (128, 128), (256, 10)]   # conv3-L1 M chunks
    m3b = [(0, 128), (128, 128)]              # conv3-L2 M chunks

    with tile.TileContext(nc) as tc:
        ctxs = []

        def pool(name, bufs):
            p = tc.tile_pool(name=name, bufs=bufs)
            ctxs.append(p)
            return p.__enter__()

        def ppool(name, bufs):
            p = tc.tile_pool(name=name, bufs=bufs, space="PSUM")
            ctxs.append(p)
            return p.__enter__()


        _tc = [0]

        def mk(pl, shape, dtype, tag):
            _tc[0] += 1
            return pl.tile(list(shape), dtype, tag=tag, name=f"{tag}_{_tc[0]}")

        pW = pool("w", 1)
        pXA = pool("xa", 2)
        pT2 = pool("t2", 1)
        pUV = pool("uv", 1)
        pSS = pool("ss", 3)
        pIDX = pool("idx", 2)
        pG = pool("gth", 2)
        pH1 = pool("h1", 2)
        pACC = pool("acc", 3)
        pH3 = pool("h3", 3)
        pG3 = pool("g3", 3)
        pSM = pool("small", 8)
        psA = ppool("psA", 1)
        psB = ppool("psB", 4)

        # load constants / weights into SBUF once (unique tag per weight:
        # they are all live for the whole kernel)
        def ld(ap, dt=None):
            _tc[0] += 1
            t = mk(pW, list(ap.shape), dt or ap.dtype, f"w{_tc[0]}")
            nc.sync.dma_start(out=t[:], in_=ap)
            return t

        onesc = ld(ones_col)
        onesr = mk(pW, [1, N], f32, "onesr")
        nc.gpsimd.memset(onesr[:], 1.0)
        W1u, W1v, W1b2, B1b = ld(w1u), ld(w1v), ld(w1b2), ld(b1b)
        W2u, W2v, W2b2, B2b = ld(w2u), ld(w2v), ld(w2b2), ld(b2b)
        W3a = [ld(a) for a, _ in w3a_k]
        W3b = [ld(a) for a, _ in w3b_k]
        B3a, B3b = ld(b3a), ld(b3b)
        WL1 = [ld(wl10), ld(wl11)]
        WL2 = [ld(wl20), ld(wl21)]
        BL1, BL2 = ld(bl1), ld(bl2)

        def lrelu(eng, out_ap, in_ap):
            # out = max(SLOPE * in, in)
            eng.scalar_tensor_tensor(out_ap, in_ap, SLOPE, in_ap,
                                     op0=Alu.mult, op1=Alu.max)

        def fill_halfsq(xrows_ap, c, dst_row_ap):
            """dst_row[0, :] = -0.5 * sum_c xrows[c, :]^2  (via ACT square +
            PE ones-matmul + ACT scaled copy)."""
            t2 = mk(pT2, [c, N], f32, "t2")
            nc.scalar.activation(t2[:], xrows_ap, Act.Square)
            hsq = mk(pT2, [1, N], f32, "hsq")
            for fc in range(FC):
                ps = mk(psB, [128, FCW], f32, "psB")
                sl = slice(fc * FCW, (fc + 1) * FCW)
                nc.tensor.matmul(ps[0:1, :], onesc[0:c, :], t2[:, sl],
                                 start=True, stop=True)
                nc.scalar.activation(hsq[:, sl], ps[0:1, :], Act.Copy,
                                     scale=-0.5)
            nc.gpsimd.dma_start(out=dst_row_ap, in_=hsq[:])

        def stage(s, xs, xm, cp1, u_t, v_t, ch, w_l2, co2, mchunks, accs,
                  l2_f16):
            """edge-conv stage: rank matrix -> topk -> gather -> conv-L2 ->
            per-kk running max into accs (list of (acc_tile, mrows))."""
            # --- rank matrix + topk scans ---
            I = mk(pIDX, [128, NT * 16], u16, "I")
            for t in range(NT):
                S = mk(psA, [128, N], f32, "psA")
                for fc in range(FC):
                    sl = slice(fc * FCW, (fc + 1) * FCW)
                    nc.tensor.matmul(S[:, sl], xs[0:cp1, t * 128:(t + 1) * 128],
                                     xm[0:cp1, sl], start=True, stop=True)
                sS = mk(pSS, [128, N], f32, "sS")
                nc.scalar.activation(sS[:], S[:], Act.Copy)
                if scan_v2:
                    # top-8 of each contiguous quarter -> 32 candidates; the
                    # global top-16 values are among them (up to the rare
                    # >=9-of-10-in-one-quarter case, which only perturbs
                    # ranks 9/10); indices via two full-width max_index.
                    NQ = N // 4
                    VC = mk(pSM, [128, 32], f32, "VC")
                    for q in range(4):
                        nc.vector.max(VC[:, 8 * q:8 * q + 8],
                                      sS[:, q * NQ:(q + 1) * NQ])
                    v8 = mk(pSM, [128, 8], f32, "v8")
                    nc.vector.max(v8[:], VC[:])
                    nc.vector.max_index(I[:, t * 16:t * 16 + 8], v8[:], sS[:])
                    nc.vector.match_replace(VC[:], v8[:], VC[:], NEG_BIG)
                    v8b = mk(pSM, [128, 8], f32, "v8")
                    nc.vector.max(v8b[:], VC[:])
                    nc.vector.max_index(I[:, t * 16 + 8:t * 16 + 16], v8b[:],
                                        sS[:])
                else:
                    v8 = mk(pSM, [128, 8], f32, "v8")
                    nc.vector.max(v8[:], sS[:])
                    nc.vector.max_index(I[:, t * 16:t * 16 + 8], v8[:], sS[:])
                    nc.vector.match_replace(sS[:], v8[:], sS[:], NEG_BIG)
                    v8b = mk(pSM, [128, 8], f32, "v8")
                    nc.vector.max(v8b[:], sS[:])
                    nc.vector.max_index(I[:, t * 16 + 8:t * 16 + 16], v8b[:],
                                        sS[:])

            # --- shuffle idx into wrapped gather list + replicate ---
            G = mk(pG, [128, KNN * NS], u16, "G")
            dstv = G[0:16, :].rearrange("p (kk t r) -> p kk t r",
                                        kk=KNN, t=NT, r=8)
            for r in range(8):
                for t in range(NT):
                    nc.gpsimd.dma_start(
                        out=dstv[:, :, t, r],
                        in_=I[16 * r:16 * (r + 1), t * 16:t * 16 + KNN])
            nc.sync.dma_start(out=G[16:32, :], in_=G[0:16, :])
            nc.sync.dma_start(out=G[32:64, :], in_=G[0:32, :])
            nc.sync.dma_start(out=G[64:128, :], in_=G[0:64, :])

            # --- per-kk: gather, +v, lrelu, conv-L2, running max ---
            for kk in range(KNN):
                g = mk(pG, [ch, N], f32, "g")
                nc.gpsimd.ap_gather(
                    g[:].unsqueeze(2), u_t[:].unsqueeze(2),
                    G[0:ch, kk * NS:(kk + 1) * NS].bitcast(i16),
                    channels=ch, num_elems=N, d=1, num_idxs=N)
                nc.gpsimd.tensor_tensor(out=g[:], in0=g[:], in1=v_t[:],
                                        op=Alu.add)
                h1 = mk(pH1, [ch, N], f16 if l2_f16 else f32, "h1")
                lrelu(nc.vector, h1[:], g[:])
                for mi, (mo, ms) in enumerate(mchunks):
                    acc, _ = accs[mi]
                    for fc in range(FC):
                        sl = slice(fc * FCW, (fc + 1) * FCW)
                        ps = mk(psB, [128, FCW], f32, "psB")
                        nc.tensor.matmul(ps[0:ms, :], w_l2[:, mo:mo + ms],
                                         h1[0:w_l2.shape[0], sl],
                                         start=True, stop=True)
                        if kk == 0:
                            nc.vector.tensor_copy(acc[:, sl], ps[0:ms, :])
                        else:
                            nc.vector.tensor_tensor(
                                out=acc[:, sl], in0=acc[:, sl],
                                in1=ps[0:ms, :], op=Alu.max)

        for s in range(nsamp):
            # ============= stage A =============
            xsA = mk(pXA, [3, N], f32, "xs")
            xmA = mk(pXA, [3, N], f32, "xm")
            nc.sync.dma_start(out=xsA[0:2, :], in_=x_d[s])
            nc.sync.dma_start(out=xmA[0:2, :], in_=x_d[s])
            nc.gpsimd.dma_start(out=xsA[2:3, :], in_=onesr[:])
            fill_halfsq(xsA[0:2, :], 2, xmA[2:3, :])

            # conv1-L1 u/v (replicated to 128 partitions)
            u1 = mk(pUV, [128, N], f32, "u")
            v1 = mk(pUV, [128, N], f32, "v")
            for dst, w, kc in ((u1, W1u, 2), (v1, W1v, 3)):
                for fc in range(FC):
                    sl = slice(fc * FCW, (fc + 1) * FCW)
                    ps = mk(psB, [128, FCW], f32, "psB")
                    nc.tensor.matmul(ps[:, :], w[0:kc, :], xsA[0:kc, sl],
                                     start=True, stop=True)
                    nc.scalar.activation(dst[:, sl], ps[:, :], Act.Copy)

            accA = mk(pACC, [22, N], f32, "acc")
            stage(s, xsA, xmA, 3, u1, v1, 128, W1b2, 22, [(0, 22)],
                  [(accA, 22)], l2_f16=False)

            # x1 = lrelu(accA + b1b) -> xs_B rows
            xsB = mk(pXA, [23, N], f32, "xs")
            xmB = mk(pXA, [23, N], f32, "xm")
            nc.scalar.activation(accA[:], accA[:], Act.Identity, bias=B1b[:])
            lrelu(nc.vector, xsB[0:22, :], accA[:])
            nc.gpsimd.dma_start(out=xsB[22:23, :], in_=onesr[:])
            nc.sync.dma_start(out=xmB[0:22, :], in_=xsB[0:22, :])
            fill_halfsq(xsB[0:22, :], 22, xmB[22:23, :])

            # conv2-L1 u/v
            u2 = mk(pUV, [112, N], f32, "u")
            v2 = mk(pUV, [112, N], f32, "v")
            for dst, w, kc in ((u2, W2u, 22), (v2, W2v, 23)):
                for fc in range(FC):
                    sl = slice(fc * FCW, (fc + 1) * FCW)
                    ps = mk(psB, [128, FCW], f32, "psB")
                    nc.tensor.matmul(ps[0:112, :], w[0:kc, :], xsB[0:kc, sl],
                                     start=True, stop=True)
                    nc.scalar.activation(dst[:, sl], ps[0:112, :], Act.Copy)

            # ============= stage B =============
            accB0 = mk(pACC, [128, N], f32, "acc")
            accB1 = mk(pACC, [128, N], f32, "acc")
            stage(s, xsB, xmB, 23, u2, v2, 112, W2b2, 256,
                  [(0, 128), (128, 128)], [(accB0, 128), (accB1, 128)],
                  l2_f16=True)

            # ============= stage C: conv3 =============
            h3 = [mk(pH3, [22, N], f16, "h3"),
                  mk(pH3, [128, N], f16, "h3"),
                  mk(pH3, [128, N], f16, "h3")]
            nc.vector.tensor_copy(h3[0][:, :], xsB[0:22, :])
            for acc, bcol, dst in ((accB0, 0, h3[1]), (accB1, 1, h3[2])):
                nc.scalar.activation(acc[:], acc[:], Act.Identity,
                                     bias=B2b[:, bcol:bcol + 1])
                lrelu(nc.vector, dst[:, :], acc[:, :])

            g3 = [mk(pG3, [128, N], f16, "g3"),
                  mk(pG3, [128, N], f16, "g3"),
                  mk(pG3, [10, N], f16, "g3")]
            for mi, (mo, ms) in enumerate(m3a):
                for fc in range(FC):
                    sl = slice(fc * FCW, (fc + 1) * FCW)
                    ps = mk(psB, [128, FCW], f32, "psB")
                    for ki, (wk, ks) in enumerate(zip(W3a, (22, 128, 128))):
                        nc.tensor.matmul(ps[0:ms, :], wk[0:ks, mo:mo + ms],
                                         h3[ki][0:ks, sl],
                                         start=(ki == 0), stop=(ki == 2))
                    tg = mk(pT2, [128, FCW], f32, "tg")
                    nc.scalar.activation(tg[0:ms, :], ps[0:ms, :],
                                         Act.Identity,
                                         bias=B3a[0:ms, mi:mi + 1])
                    lrelu(nc.vector, g3[mi][0:ms, sl], tg[0:ms, :])

            # conv3-L2 + global max over N
            feat = [mk(pSM, [128, 1], f16, "feat"),
                    mk(pSM, [128, 1], f16, "feat")]
            for mi, (mo, ms) in enumerate(m3b):
                part = mk(pSM, [128, FC], f32, "part")
                for fc in range(FC):
                    sl = slice(fc * FCW, (fc + 1) * FCW)
                    ps = mk(psB, [128, FCW], f32, "psB")
                    for ki, (wk, ks) in enumerate(zip(W3b, (128, 128, 10))):
                        nc.tensor.matmul(ps[0:ms, :], wk[0:ks, mo:mo + ms],
                                         g3[ki][0:ks, sl],
                                         start=(ki == 0), stop=(ki == 2))
                    nc.vector.tensor_reduce(part[0:ms, fc:fc + 1], ps[0:ms, :],
                                            axis=mybir.AxisListType.X,
                                            op=Alu.max)
                mx = mk(pSM, [128, 1], f32, "mx")
                nc.vector.tensor_reduce(mx[0:ms, :], part[0:ms, :],
                                        axis=mybir.AxisListType.X, op=Alu.max)
                tf = mk(pSM, [128, 1], f32, "tf")
                nc.scalar.activation(tf[0:ms, :], mx[0:ms, :], Act.Identity,
                                     bias=B3b[0:ms, mi:mi + 1])
                lrelu(nc.vector, feat[mi][0:ms, :], tf[0:ms, :])

            # ============= stage D: MLP + log_softmax =============
            z1 = [mk(pSM, [128, 1], f16, "z1"),
                  mk(pSM, [20, 1], f16, "z1b")]
            for mi, (mo, ms) in enumerate(((0, 128), (128, 20))):
                ps = mk(psB, [128, FCW], f32, "psB")
                for ki in range(2):
                    nc.tensor.matmul(ps[0:ms, 0:1], WL1[ki][:, mo:mo + ms],
                                     feat[ki][:, :], start=(ki == 0),
                                     stop=(ki == 1))
                tz = mk(pSM, [128, 1], f32, "tz")
                nc.scalar.activation(tz[0:ms, :], ps[0:ms, 0:1], Act.Identity,
                                     bias=BL1[0:ms, mi:mi + 1])
                lrelu(nc.vector, z1[mi][0:ms, :], tz[0:ms, :])

            ps = mk(psB, [128, FCW], f32, "psB")
            nc.tensor.matmul(ps[0:40, 0:1], WL2[0][:, :], z1[0][:, :],
                             start=True, stop=False)
            nc.tensor.matmul(ps[0:40, 0:1], WL2[1][0:20, :], z1[1][0:20, :],
                             start=False, stop=True)
            z2 = mk(pSM, [40, 1], f32, "z2")
            nc.scalar.activation(z2[:], ps[0:40, 0:1], Act.Identity,
                                 bias=BL2[:])

            zrow = mk(pSM, [1, 40], f32, "zrow")
            nc.sync.dma_start(out=zrow[:], in_=z2[:])
            m1 = mk(pSM, [1, 1], f32, "m1")
            nc.vector.tensor_reduce(m1[:], zrow[:], axis=mybir.AxisListType.X,
                                    op=Alu.max)
            mneg = mk(pSM, [1, 1], f32, "mneg")
            nc.vector.tensor_scalar_mul(mneg[:], m1[:], -1.0)
            e = mk(pSM, [1, 40], f32, "e")
            sume = mk(pSM, [1, 1], f32, "sume")
            nc.scalar.activation(e[:], zrow[:], Act.Exp, bias=mneg[:],
                                 accum_out=sume[:])
            lse = mk(pSM, [1, 1], f32, "lse")
            nc.scalar.activation(lse[:], sume[:], Act.Ln)
            res = mk(pSM, [1, 40], f32, "res")
            nc.vector.tensor_scalar(res[:], zrow[:], m1[:], lse[:],
                                    op0=Alu.subtract, op1=Alu.subtract)
            nc.sync.dma_start(out=out_d[s:s + 1, :], in_=res[:])

        for p in reversed(ctxs):
            p.__exit__(None, None, None)

    nc.compile()
    return nc


# ---------------------------------------------------------------------------
# host-side weight folding
# ---------------------------------------------------------------------------


def _fold(params):
    """(W, b, g, be) -> (W', b') with eval-BN folded (running stats 0/1)."""
    out = []
    for (W, b, g, be) in params:
        W = np.asarray(W, np.float32)
        b = np.asarray(b, np.float32)
        g = np.asarray(g, np.float32)
        be = np.asarray(be, np.float32)
        sc = g / np.sqrt(np.float32(1.0) + np.float32(EPS))
        out.append((W * sc[:, None], b * sc + be))
    return out


def _pad(a, shape):
    out = np.zeros(shape, a.dtype)
    out[tuple(slice(0, s) for s in a.shape)] = a
    return out


def make_weight_map(conv1_params, conv2_params, conv3_params, lin_params):
    f32, f16 = np.float32, np.float16
    (W1a, b1a), (W1b, b1b) = _fold(conv1_params)
    (W2a, b2a), (W2b, b2b) = _fold(conv2_params)
    (W3a, b3a), (W3b, b3b) = _fold(conv3_params)
    lf = _fold(lin_params)
    (L1, bl1), (L2, bl2) = lf[0], lf[1]
    # NOTE: reference _mlp applies BN only to non-final layers; final layer's
    # BN params are unused there, but _fold(L2) with g=1, be=0 is identity
    # anyway; to be exact, recompute final layer unfolded:
    L2 = np.asarray(lin_params[1][0], f32)
    bl2 = np.asarray(lin_params[1][1], f32)

    d = {}
    d["ones_col"] = np.ones((128, 1), f32)
    # conv1-L1: u = Wf' x ; v = (Wc'-Wf') x + b'
    Au = W1a[:, 0:2].T            # [2, 9]
    Av = (W1a[:, 2:4] - W1a[:, 0:2]).T
    d["w1u"] = np.tile(_pad(Au, (2, 16)), (1, 8)).astype(f32)
    w1v = np.zeros((3, 128), f32)
    w1v[0:2] = np.tile(_pad(Av, (2, 16)), (1, 8))
    w1v[2] = np.tile(_pad(b1a[None, :], (1, 16)), (1, 8))[0]
    d["w1v"] = w1v
    d["w1b2"] = _pad(W1b.T, (16, 22)).astype(f32)   # [9->16, 22]
    d["b1b"] = b1b[:, None].astype(f32)
    # conv2-L1
    Au2 = W2a[:, 0:22].T          # [22, 104]
    Av2 = (W2a[:, 22:44] - W2a[:, 0:22]).T
    d["w2u"] = _pad(Au2, (22, 112)).astype(f32)
    w2v = np.zeros((23, 112), f32)
    w2v[0:22] = _pad(Av2, (22, 112))
    w2v[22, 0:104] = b2a
    d["w2v"] = w2v
    d["w2b2"] = _pad(W2b.T, (112, 256)).astype(f16)  # [104->112, 256]
    d["b2b"] = b2b.reshape(2, 128).T.copy().astype(f32)
    # conv3
    W3aT = W3a.T                  # [278, 266]
    d["w3a0"] = W3aT[0:22].astype(f16)
    d["w3a1"] = W3aT[22:150].astype(f16)
    d["w3a2"] = W3aT[150:278].astype(f16)
    b3a_t = np.zeros((128, 3), f32)
    b3a_t[0:128, 0] = b3a[0:128]
    b3a_t[0:128, 1] = b3a[128:256]
    b3a_t[0:10, 2] = b3a[256:266]
    d["b3a"] = b3a_t
    W3bT = W3b.T                  # [266, 256]
    d["w3b0"] = W3bT[0:128].astype(f16)
    d["w3b1"] = W3bT[128:256].astype(f16)
    d["w3b2"] = W3bT[256:266].astype(f16)
    d["b3b"] = b3b.reshape(2, 128).T.copy().astype(f32)
    # mlp
    L1T = L1.T                    # [256, 148]
    d["wl10"] = L1T[0:128].astype(f16)
    d["wl11"] = L1T[128:256].astype(f16)
    bl1_t = np.zeros((128, 2), f32)
    bl1_t[0:128, 0] = bl1[0:128]
    bl1_t[0:20, 1] = bl1[128:148]
    d["bl1"] = bl1_t
    L2T = L2.T                    # [148, 40]
    d["wl20"] = L2T[0:128].astype(f16)
    d["wl21"] = L2T[128:148].astype(f16)
    d["bl2"] = bl2[:, None].astype(f32)
    return d


# ---------------------------------------------------------------------------
# public entry point
# ---------------------------------------------------------------------------

_LAST_RESULTS = {}


def kernel(x, conv1_params, conv2_params, conv3_params, lin_params):
    from concourse.bass_utils import run_bass_kernel_spmd

    x = np.asarray(x, np.float32)
    B, C, N = x.shape
    n_cores = 8
    nsamp = B // n_cores
    wmap = make_weight_map(conv1_params, conv2_params, conv3_params,
                           lin_params)
    nc = _build_program(nsamp, N, SCAN_V2)
    in_maps = [
        {"x": x[c * nsamp:(c + 1) * nsamp], **wmap} for c in range(n_cores)
    ]
    trace = bool(int(os.environ.get("KERNEL_TRACE", "0")))
    r = run_bass_kernel_spmd(nc, in_maps, list(range(n_cores)), trace=trace)
    _LAST_RESULTS["exec_time_ns"] = getattr(r, "exec_time_ns", None)
    out = np.concatenate([res["out"] for res in r.results], axis=0)
    return out.astype(np.float32)


# revision 20
# speedup vs baseline: 1.8705x; 1.8705x over previous
"""DGCNN (nn_DGCNN_70643622084664) Trainium2 Bass kernel.

Data-parallel over batch: B=16 samples sharded 2-per-core across 8 NeuronCores.
Per sample pipeline (all on device):
  stage A: pairwise -dist^2 ranking matrix via augmented matmul -> top-10 via
           DVE max8/max_index/match_replace -> gpsimd ap_gather of conv1-L1
           pre-activations -> +v, LeakyReLU -> conv1-L2 matmul -> max over k
  stage B: same with C=22 -> x2
  stage C: conv3 (two 1x1 conv layers) on [x1;x2] -> global max over N
  stage D: MLP + log_softmax

BN (eval, running stats 0/1) is folded into weights/biases on the host.
The first layer of each edge-conv is decomposed as
  W @ [feat-center; center] = u_gather + v,  u = Wf' x,  v = (Wc'-Wf') x + b'
so the k-NN gather happens after the (cheap) pointwise conv.
"""

import functools
import os

SCAN_V2 = bool(int(os.environ.get("KERNEL_SCANV2", "1")))
FP32R = bool(int(os.environ.get("KERNEL_FP32R", "0")))

import numpy as np

EPS = 1e-5
SLOPE = 0.2
KNN = 10
NEG_BIG = -3.0e38

# ---------------------------------------------------------------------------
# program builder
# ---------------------------------------------------------------------------


@functools.lru_cache(maxsize=None)
def _build_program(nsamp: int, n_pts: int, scan_v2: bool = True,
                   fp32r: bool = False):
    import concourse.bacc as bacc
    import concourse.bass as bass
    import concourse.mybir as mybir
    from concourse import tile

    f32 = mybir.dt.float32
    f16 = mybir.dt.float16
    u16 = mybir.dt.uint16
    i16 = mybir.dt.int16
    Alu = mybir.AluOpType
    Act = mybir.ActivationFunctionType

    N = n_pts
    NT = N // 128
    NS = N // 16          # gather-list slots per kk
    FC = N // 512 if N >= 512 else 1   # 512-wide free chunks per full row
    FCW = min(512, N)

    nc = bacc.Bacc("TRN2", target_bir_lowering=False, debug=False)

    def din(name, shape, dt=f32):
        return nc.dram_tensor(name, list(shape), dt, kind="ExternalInput").ap()

    # inputs -----------------------------------------------------------------
    x_d = din("x", [nsamp, 2, N])
    ones_col = din("ones_col", [128, 1])
    ones_row = din("ones_row", [1, n_pts])
    w1u = din("w1u", [2, 128])          # conv1-L1 u lhsT (replicated x8)
    w1v = din("w1v", [3, 128])          # conv1-L1 v lhsT (+bias row)
    w1b2 = din("w1b2", [16, 22])        # conv1-L2 lhsT  (fp32)
    b1b = din("b1b", [22, 1])
    w2u = din("w2u", [22, 112])
    w2v = din("w2v", [23, 112])
    w2b2 = din("w2b2", [112, 256], f16)  # conv2-L2 lhsT, M split 2x128
    b2b = din("b2b", [128, 2])
    w3a0 = din("w3a0", [22, 266], f16)   # conv3-L1 lhsT K-tiles (x1 | x2a | x2b)
    w3a1 = din("w3a1", [128, 266], f16)
    w3a2 = din("w3a2", [128, 266], f16)
    b3a = din("b3a", [128, 3])
    w3b0 = din("w3b0", [128, 256], f16)  # conv3-L2 lhsT K-tiles
    w3b1 = din("w3b1", [128, 256], f16)
    w3b2 = din("w3b2", [10, 256], f16)
    b3b = din("b3b", [128, 2])
    wl10 = din("wl10", [128, 148], f16)
    wl11 = din("wl11", [128, 148], f16)
    bl1 = din("bl1", [128, 2])
    wl20 = din("wl20", [128, 40], f16)
    wl21 = din("wl21", [20, 40], f16)
    bl2 = din("bl2", [40, 1])

    out_d = nc.dram_tensor("out", [nsamp, 40], f32, kind="ExternalOutput").ap()

    w3a_k = [(w3a0, 22), (w3a1, 128), (w3a2, 128)]
    w3b_k = [(w3b0, 128), (w3b1, 128), (w3b2, 10)]
    m3a = [(0, 128), (128, 128), (256, 10)]   # conv3-L1 M chunks
    m3b = [(0, 128), (128, 128)]              # conv3-L2 M chunks

    with tile.TileContext(nc) as tc:
        ctxs = []

        def pool(name, bufs):
            p = tc.tile_pool(name=name, bufs=bufs)
            ctxs.append(p)
            return p.__enter__()

        def ppool(name, bufs):
            p = tc.tile_pool(name=name, bufs=bufs, space="PSUM")
            ctxs.append(p)
            return p.__enter__()


        _tc = [0]

        def mk(pl, shape, dtype, tag):
            _tc[0] += 1
            return pl.tile(list(shape), dtype, tag=tag, name=f"{tag}_{_tc[0]}")

        pW = pool("w", 1)
        pXA = pool("xa", 3)
        pXM = pool("xam", 2)
        pT2 = pool("t2", 1)
        pUV = pool("uv", 2)
        pSS = pool("ss", 2)
        pIDX = pool("idx", 2)
        pG = pool("gth", 2)
        pH1 = pool("h1", 2)
        pACC = pool("acc", 4)
        pH3 = pool("h3", 3)
        pG3 = pool("g3", 3)
        pSM = pool("small", 2)
        psA = ppool("psA", 1)
        psB = ppool("psB", 4)

        # load constants / weights into SBUF once (unique tag per weight:
        # they are all live for the whole kernel)
        def ld(ap, dt=None):
            _tc[0] += 1
            t = mk(pW, list(ap.shape), dt or ap.dtype, f"w{_tc[0]}")
            nc.sync.dma_start(out=t[:], in_=ap)
            return t

        onesc = ld(ones_col)
        W1u, W1v, W1b2, B1b = ld(w1u), ld(w1v), ld(w1b2), ld(b1b)
        W2u, W2v, W2b2, B2b = ld(w2u), ld(w2v), ld(w2b2), ld(b2b)
        W3a = [ld(a) for a, _ in w3a_k]
        W3b = [ld(a) for a, _ in w3b_k]
        B3a, B3b = ld(b3a), ld(b3b)
        WL1 = [ld(wl10), ld(wl11)]
        WL2 = [ld(wl20), ld(wl21)]
        BL1, BL2 = ld(bl1), ld(bl2)

        def lrelu(eng, out_ap, in_ap):
            # out = max(SLOPE * in, in)
            eng.scalar_tensor_tensor(out_ap, in_ap, SLOPE, in_ap,
                                     op0=Alu.mult, op1=Alu.max)

        def fill_halfsq(xrows_ap, c, dst_row_ap):
            """dst_row[0, :] = -0.5 * sum_c xrows[c, :]^2  (via ACT square +
            PE ones-matmul + ACT scaled copy)."""
            t2 = mk(pT2, [c, N], f32, "t2")
            nc.scalar.activation(t2[:], xrows_ap, Act.Square)
            hsq = mk(pT2, [1, N], f32, "t2")
            for fc in range(FC):
                ps = mk(psB, [128, FCW], f32, "psB")
                sl = slice(fc * FCW, (fc + 1) * FCW)
                nc.tensor.matmul(ps[0:1, :], onesc[0:c, :], t2[:, sl],
                                 start=True, stop=True)
                nc.scalar.activation(hsq[:, sl], ps[0:1, :], Act.Copy,
                                     scale=-0.5)
            nc.gpsimd.dma_start(out=dst_row_ap, in_=hsq[:])

        f32r = mybir.dt.float32r

        def stage_scans(xs, xm, cp1):
            """rank matrix -> topk indices -> wrapped gather list G."""
            I = mk(pIDX, [128, NT * 16], u16, "I")
            for t in range(NT):
                S = mk(psA, [128, N], f32, "psA")
                for fc in range(FC):
                    sl = slice(fc * FCW, (fc + 1) * FCW)
                    lhsT = xs[0:cp1, t * 128:(t + 1) * 128]
                    rhs = xm[0:cp1, sl]
                    if fp32r:
                        lhsT = lhsT.bitcast(f32r)
                        rhs = rhs.bitcast(f32r)
                    nc.tensor.matmul(S[:, sl], lhsT, rhs,
                                     start=True, stop=True)
                sS = mk(pSS, [128, N], f32, "sS")
                nc.scalar.activation(sS[:], S[:], Act.Copy)
                if scan_v2:
                    # top-8 of each contiguous quarter -> 32 candidates; the
                    # global top-16 values are among them (up to the rare
                    # >=9-of-10-in-one-quarter case, which only perturbs
                    # ranks 9/10); indices via two full-width max_index.
                    NQ = N // 4
                    VC = mk(pSM, [128, 32], f32, "VC")
                    for q in range(4):
                        nc.vector.max(VC[:, 8 * q:8 * q + 8],
                                      sS[:, q * NQ:(q + 1) * NQ])
                    v8 = mk(pSM, [128, 8], f32, "v8")
                    nc.vector.max(v8[:], VC[:])
                    nc.vector.max_index(I[:, t * 16:t * 16 + 8], v8[:], sS[:])
                    nc.vector.match_replace(VC[:], v8[:], VC[:], NEG_BIG)
                    v8b = mk(pSM, [128, 8], f32, "v8")
                    nc.vector.max(v8b[:], VC[:])
                    nc.vector.max_index(I[:, t * 16 + 8:t * 16 + 16], v8b[:],
                                        sS[:])
                else:
                    v8 = mk(pSM, [128, 8], f32, "v8")
                    nc.vector.max(v8[:], sS[:])
                    nc.vector.max_index(I[:, t * 16:t * 16 + 8], v8[:], sS[:])
                    nc.vector.match_replace(sS[:], v8[:], sS[:], NEG_BIG)
                    v8b = mk(pSM, [128, 8], f32, "v8")
                    nc.vector.max(v8b[:], sS[:])
                    nc.vector.max_index(I[:, t * 16 + 8:t * 16 + 16], v8b[:],
                                        sS[:])

            # --- shuffle idx into wrapped gather list + replicate ---
            G = mk(pG, [128, KNN * NS], u16, "G")
            dstv = G[0:16, :].rearrange("p (kk t r) -> p kk t r",
                                        kk=KNN, t=NT, r=8)
            for r in range(8):
                for t in range(NT):
                    nc.gpsimd.dma_start(
                        out=dstv[:, :, t, r],
                        in_=I[16 * r:16 * (r + 1), t * 16:t * 16 + KNN])
            nc.sync.dma_start(out=G[16:32, :], in_=G[0:16, :])
            nc.sync.dma_start(out=G[32:64, :], in_=G[0:32, :])
            nc.sync.dma_start(out=G[64:128, :], in_=G[0:64, :])
            return G

        def stage_gather(G, u_t, v_t, ch, w_l2, mchunks, accs, l2_f16):
            """per-kk: gather, +v, lrelu, conv-L2, running max into accs."""
            for kk in range(KNN):
                g = mk(pG, [ch, N], f32, "g")
                nc.gpsimd.ap_gather(
                    g[:].unsqueeze(2), u_t[:].unsqueeze(2),
                    G[0:ch, kk * NS:(kk + 1) * NS].bitcast(i16),
                    channels=ch, num_elems=N, d=1, num_idxs=N)
                nc.gpsimd.tensor_tensor(out=g[:], in0=g[:], in1=v_t[:],
                                        op=Alu.add)
                h1 = mk(pH1, [ch, N], f16 if l2_f16 else f32, "h1")
                lrelu(nc.vector, h1[:], g[:])
                for mi, (mo, ms) in enumerate(mchunks):
                    acc, _ = accs[mi]
                    for fc in range(FC):
                        sl = slice(fc * FCW, (fc + 1) * FCW)
                        ps = mk(psB, [128, FCW], f32, "psB")
                        nc.tensor.matmul(ps[0:ms, :], w_l2[:, mo:mo + ms],
                                         h1[0:w_l2.shape[0], sl],
                                         start=True, stop=True)
                        if kk == 0:
                            nc.vector.tensor_copy(acc[:, sl], ps[0:ms, :])
                        else:
                            nc.vector.tensor_tensor(
                                out=acc[:, sl], in0=acc[:, sl],
                                in1=ps[0:ms, :], op=Alu.max)

        def sample_phases(s):
            st = {}

            def p0():
                # stage-A prep: xs/xm + conv1-L1 u/v
                xsA = mk(pXA, [3, N], f32, "xs")
                xmA = mk(pXM, [3, N], f32, "xm")
                nc.sync.dma_start(out=xsA[0:2, :], in_=x_d[s])
                nc.sync.dma_start(out=xmA[0:2, :], in_=x_d[s])
                nc.gpsimd.dma_start(out=xsA[2:3, :], in_=ones_row)
                fill_halfsq(xsA[0:2, :], 2, xmA[2:3, :])
                u1 = mk(pUV, [128, N], f32, "u")
                v1 = mk(pUV, [128, N], f32, "v")
                for dst, w, kc in ((u1, W1u, 2), (v1, W1v, 3)):
                    for fc in range(FC):
                        sl = slice(fc * FCW, (fc + 1) * FCW)
                        ps = mk(psB, [128, FCW], f32, "psB")
                        nc.tensor.matmul(ps[:, :], w[0:kc, :], xsA[0:kc, sl],
                                         start=True, stop=True)
                        nc.scalar.activation(dst[:, sl], ps[:, :], Act.Copy)
                st.update(xsA=xsA, xmA=xmA, u1=u1, v1=v1)

            def p1():
                st["GA"] = stage_scans(st["xsA"], st["xmA"], 3)

            def p2():
                accA = mk(pACC, [22, N], f32, "acc")
                stage_gather(st["GA"], st["u1"], st["v1"], 128, W1b2,
                             [(0, 22)], [(accA, 22)], l2_f16=False)
                # x1 = lrelu(accA + b1b) -> xs_B rows; stage-B prep
                xsB = mk(pXA, [23, N], f32, "xs")
                xmB = mk(pXM, [23, N], f32, "xm")
                nc.scalar.activation(accA[:], accA[:], Act.Identity,
                                     bias=B1b[:])
                lrelu(nc.vector, xsB[0:22, :], accA[:])
                nc.gpsimd.dma_start(out=xsB[22:23, :], in_=ones_row)
                nc.sync.dma_start(out=xmB[0:22, :], in_=xsB[0:22, :])
                fill_halfsq(xsB[0:22, :], 22, xmB[22:23, :])
                st.update(xsB=xsB, xmB=xmB)

            def p3():
                st["GB"] = stage_scans(st["xsB"], st["xmB"], 23)
                xsB = st["xsB"]
                u2 = mk(pUV, [112, N], f32, "u")
                v2 = mk(pUV, [112, N], f32, "v")
                for dst, w, kc in ((u2, W2u, 22), (v2, W2v, 23)):
                    for fc in range(FC):
                        sl = slice(fc * FCW, (fc + 1) * FCW)
                        ps = mk(psB, [128, FCW], f32, "psB")
                        nc.tensor.matmul(ps[0:112, :], w[0:kc, :],
                                         xsB[0:kc, sl], start=True, stop=True)
                        nc.scalar.activation(dst[:, sl], ps[0:112, :],
                                             Act.Copy)
                st.update(u2=u2, v2=v2)

            def p4():
                accB0 = mk(pACC, [128, N], f32, "acc")
                accB1 = mk(pACC, [128, N], f32, "acc")
                stage_gather(st["GB"], st["u2"], st["v2"], 112, W2b2,
                             [(0, 128), (128, 128)],
                             [(accB0, 128), (accB1, 128)], l2_f16=True)
                st.update(accB0=accB0, accB1=accB1)

            def p5():
                accB0, accB1, xsB = st["accB0"], st["accB1"], st["xsB"]
                h3 = [mk(pH3, [22, N], f16, "h3"),
                      mk(pH3, [128, N], f16, "h3"),
                      mk(pH3, [128, N], f16, "h3")]
                nc.vector.tensor_copy(h3[0][:, :], xsB[0:22, :])
                for acc, bcol, dst in ((accB0, 0, h3[1]), (accB1, 1, h3[2])):
                    nc.scalar.activation(acc[:], acc[:], Act.Identity,
                                         bias=B2b[:, bcol:bcol + 1])
                    lrelu(nc.vector, dst[:, :], acc[:, :])

                g3 = [mk(pG3, [128, N], f16, "g3"),
                      mk(pG3, [128, N], f16, "g3"),
                      mk(pG3, [10, N], f16, "g3")]
                for mi, (mo, ms) in enumerate(m3a):
                    for fc in range(FC):
                        sl = slice(fc * FCW, (fc + 1) * FCW)
                        ps = mk(psB, [128, FCW], f32, "psB")
                        for ki, (wk, ks) in enumerate(zip(W3a,
                                                          (22, 128, 128))):
                            nc.tensor.matmul(ps[0:ms, :],
                                             wk[0:ks, mo:mo + ms],
                                             h3[ki][0:ks, sl],
                                             start=(ki == 0), stop=(ki == 2))
                        tg = mk(pT2, [128, FCW], f32, "tg")
                        nc.scalar.activation(tg[0:ms, :], ps[0:ms, :],
                                             Act.Identity,
                                             bias=B3a[0:ms, mi:mi + 1])
                        lrelu(nc.vector, g3[mi][0:ms, sl], tg[0:ms, :])

                feat = [mk(pSM, [128, 1], f16, "feat"),
                        mk(pSM, [128, 1], f16, "feat")]
                for mi, (mo, ms) in enumerate(m3b):
                    part = mk(pSM, [128, FC], f32, "part")
                    for fc in range(FC):
                        sl = slice(fc * FCW, (fc + 1) * FCW)
                        ps = mk(psB, [128, FCW], f32, "psB")
                        for ki, (wk, ks) in enumerate(zip(W3b,
                                                          (128, 128, 10))):
                            nc.tensor.matmul(ps[0:ms, :],
                                             wk[0:ks, mo:mo + ms],
                                             g3[ki][0:ks, sl],
                                             start=(ki == 0), stop=(ki == 2))
                        nc.vector.tensor_reduce(part[0:ms, fc:fc + 1],
                                                ps[0:ms, :],
                                                axis=mybir.AxisListType.X,
                                                op=Alu.max)
                    mx = mk(pSM, [128, 1], f32, "mx")
                    nc.vector.tensor_reduce(mx[0:ms, :], part[0:ms, :],
                                            axis=mybir.AxisListType.X,
                                            op=Alu.max)
                    tf = mk(pSM, [128, 1], f32, "tf")
                    nc.scalar.activation(tf[0:ms, :], mx[0:ms, :],
                                         Act.Identity,
                                         bias=B3b[0:ms, mi:mi + 1])
                    lrelu(nc.vector, feat[mi][0:ms, :], tf[0:ms, :])

                z1 = [mk(pSM, [128, 1], f16, "z1"),
                      mk(pSM, [20, 1], f16, "z1b")]
                for mi, (mo, ms) in enumerate(((0, 128), (128, 20))):
                    ps = mk(psB, [128, FCW], f32, "psB")
                    for ki in range(2):
                        nc.tensor.matmul(ps[0:ms, 0:1],
                                         WL1[ki][:, mo:mo + ms],
                                         feat[ki][:, :], start=(ki == 0),
                                         stop=(ki == 1))
                    tz = mk(pSM, [128, 1], f32, "tz")
                    nc.scalar.activation(tz[0:ms, :], ps[0:ms, 0:1],
                                         Act.Identity,
                                         bias=BL1[0:ms, mi:mi + 1])
                    lrelu(nc.vector, z1[mi][0:ms, :], tz[0:ms, :])

                ps = mk(psB, [128, FCW], f32, "psB")
                nc.tensor.matmul(ps[0:40, 0:1], WL2[0][:, :], z1[0][:, :],
                                 start=True, stop=False)
                nc.tensor.matmul(ps[0:40, 0:1], WL2[1][0:20, :],
                                 z1[1][0:20, :], start=False, stop=True)
                z2 = mk(pSM, [40, 1], f32, "z2")
                nc.scalar.activation(z2[:], ps[0:40, 0:1], Act.Identity,
                                     bias=BL2[:])

                zrow = mk(pSM, [1, 40], f32, "zrow")
                nc.sync.dma_start(out=zrow[:], in_=z2[:])
                m1 = mk(pSM, [1, 1], f32, "m1")
                nc.vector.tensor_reduce(m1[:], zrow[:],
                                        axis=mybir.AxisListType.X, op=Alu.max)
                mneg = mk(pSM, [1, 1], f32, "mneg")
                nc.vector.tensor_scalar_mul(mneg[:], m1[:], -1.0)
                e = mk(pSM, [1, 40], f32, "e")
                sume = mk(pSM, [1, 1], f32, "sume")
                nc.scalar.activation(e[:], zrow[:], Act.Exp, bias=mneg[:],
                                     accum_out=sume[:])
                lse = mk(pSM, [1, 1], f32, "lse")
                nc.scalar.activation(lse[:], sume[:], Act.Ln)
                res = mk(pSM, [1, 40], f32, "res")
                nc.vector.tensor_scalar(res[:], zrow[:], m1[:], lse[:],
                                        op0=Alu.subtract, op1=Alu.subtract)
                nc.sync.dma_start(out=out_d[s:s + 1, :], in_=res[:])

            return [p0, p1, p2, p3, p4, p5]

        # software-pipeline samples with a 2-phase skew so one sample's
        # DVE scan phase overlaps the other's gpsimd gather phase
        phases = [sample_phases(s) for s in range(nsamp)]
        NPH = 6
        SKEW = 1
        for step in range(NPH + SKEW * (nsamp - 1)):
            for s in range(nsamp):
                idx = step - SKEW * s
                if 0 <= idx < NPH:
                    phases[s][idx]()

        for p in reversed(ctxs):
            p.__exit__(None, None, None)

    nc.compile()
    return nc


# ---------------------------------------------------------------------------
# host-side weight folding
# ---------------------------------------------------------------------------


def _fold(params):
    """(W, b, g, be) -> (W', b') with eval-BN folded (running stats 0/1)."""
    out = []
    for (W, b, g, be) in params:
        W = np.asarray(W, np.float32)
        b = np.asarray(b, np.float32)
        g = np.asarray(g, np.float32)
        be = np.asarray(be, np.float32)
        sc = g / np.sqrt(np.float32(1.0) + np.float32(EPS))
        out.append((W * sc[:, None], b * sc + be))
    return out


def _pad(a, shape):
    out = np.zeros(shape, a.dtype)
    out[tuple(slice(0, s) for s in a.shape)] = a
    return out


def make_weight_map(conv1_params, conv2_params, conv3_params, lin_params,
                    n_pts=2048):
    f32, f16 = np.float32, np.float16
    (W1a, b1a), (W1b, b1b) = _fold(conv1_params)
    (W2a, b2a), (W2b, b2b) = _fold(conv2_params)
    (W3a, b3a), (W3b, b3b) = _fold(conv3_params)
    lf = _fold(lin_params)
    (L1, bl1), (L2, bl2) = lf[0], lf[1]
    # NOTE: reference _mlp applies BN only to non-final layers; final layer's
    # BN params are unused there, but _fold(L2) with g=1, be=0 is identity
    # anyway; to be exact, recompute final layer unfolded:
    L2 = np.asarray(lin_params[1][0], f32)
    bl2 = np.asarray(lin_params[1][1], f32)

    d = {}
    d["ones_col"] = np.ones((128, 1), f32)
    d["ones_row"] = np.ones((1, n_pts), f32)
    # conv1-L1: u = Wf' x ; v = (Wc'-Wf') x + b'
    Au = W1a[:, 0:2].T            # [2, 9]
    Av = (W1a[:, 2:4] - W1a[:, 0:2]).T
    d["w1u"] = np.tile(_pad(Au, (2, 16)), (1, 8)).astype(f32)
    w1v = np.zeros((3, 128), f32)
    w1v[0:2] = np.tile(_pad(Av, (2, 16)), (1, 8))
    w1v[2] = np.tile(_pad(b1a[None, :], (1, 16)), (1, 8))[0]
    d["w1v"] = w1v
    d["w1b2"] = _pad(W1b.T, (16, 22)).astype(f32)   # [9->16, 22]
    d["b1b"] = b1b[:, None].astype(f32)
    # conv2-L1
    Au2 = W2a[:, 0:22].T          # [22, 104]
    Av2 = (W2a[:, 22:44] - W2a[:, 0:22]).T
    d["w2u"] = _pad(Au2, (22, 112)).astype(f32)
    w2v = np.zeros((23, 112), f32)
    w2v[0:22] = _pad(Av2, (22, 112))
    w2v[22, 0:104] = b2a
    d["w2v"] = w2v
    d["w2b2"] = _pad(W2b.T, (112, 256)).astype(f16)  # [104->112, 256]
    d["b2b"] = b2b.reshape(2, 128).T.copy().astype(f32)
    # conv3
    W3aT = W3a.T                  # [278, 266]
    d["w3a0"] = W3aT[0:22].astype(f16)
    d["w3a1"] = W3aT[22:150].astype(f16)
    d["w3a2"] = W3aT[150:278].astype(f16)
    b3a_t = np.zeros((128, 3), f32)
    b3a_t[0:128, 0] = b3a[0:128]
    b3a_t[0:128, 1] = b3a[128:256]
    b3a_t[0:10, 2] = b3a[256:266]
    d["b3a"] = b3a_t
    W3bT = W3b.T                  # [266, 256]
    d["w3b0"] = W3bT[0:128].astype(f16)
    d["w3b1"] = W3bT[128:256].astype(f16)
    d["w3b2"] = W3bT[256:266].astype(f16)
    d["b3b"] = b3b.reshape(2, 128).T.copy().astype(f32)
    # mlp
    L1T = L1.T                    # [256, 148]
    d["wl10"] = L1T[0:128].astype(f16)
    d["wl11"] = L1T[128:256].astype(f16)
    bl1_t = np.zeros((128, 2), f32)
    bl1_t[0:128, 0] = bl1[0:128]
    bl1_t[0:20, 1] = bl1[128:148]
    d["bl1"] = bl1_t
    L2T = L2.T                    # [148, 40]
    d["wl20"] = L2T[0:128].astype(f16)
    d["wl21"] = L2T[128:148].astype(f16)
    d["bl2"] = bl2[:, None].astype(f32)
    return d


# ---------------------------------------------------------------------------
# public entry point
# ---------------------------------------------------------------------------

_LAST_RESULTS = {}


def kernel(x, conv1_params, conv2_params, conv3_params, lin_params):
    from concourse.bass_utils import run_bass_kernel_spmd

    x = np.asarray(x, np.float32)
    B, C, N = x.shape
    n_cores = 8
    nsamp = B // n_cores
    wmap = make_weight_map(conv1_params, conv2_params, conv3_params,
                           lin_params)
    nc = _build_program(nsamp, N, SCAN_V2, FP32R)
    in_maps = [
        {"x": x[c * nsamp:(c + 1) * nsamp], **wmap} for c in range(n_cores)
    ]
    trace = bool(int(os.environ.get("KERNEL_TRACE", "0")))
    r = run_bass_kernel_spmd(nc, in_maps, list(range(n_cores)), trace=trace)
    _LAST_RESULTS["exec_time_ns"] = getattr(r, "exec_time_ns", None)
    out = np.concatenate([res["out"] for res in r.results], axis=0)
    return out.astype(np.float32)
